# revision 2
# baseline (speedup 1.0000x reference)
"""BIMPM forward kernel for 8 Trainium2 NeuronCores.

Strategy (per sharding hint): data-parallel over batch. The 32-element batch
is split into 8 shards of 4; embedding/LSTM/matching/aggregation weights are
replicated on every core; all matching ops are batch-local, so there is no
cross-core communication. The forward pass is split into three pmapped
phases (context BiLSTM / multi-perspective matching / aggregation BiLSTM +
classifier head) compiled separately for the neuron backend; any phase whose
device compile fails falls back to host execution so the kernel always
produces correct output. The final (32,2) argmax runs on host.

Hardcoded problem dims: B=32, S=64, V=40000, E=300, H=256, L=20, C=2.
"""

import numpy as np

B, S, V, E, H, L, C = 32, 64, 40000, 300, 256, 20, 2
N_CORES = 8
EPS = 1e-8

_STATE = {}


def _build():
    import jax
    import jax.numpy as jnp

    def _div(n, d):
        d = jnp.where(d > EPS, d, EPS)
        return n / d

    def _lstm(x, wi, wh, bi, bh, reverse=False):
        if reverse:
            x = x[:, ::-1]
        xg = jnp.einsum('bsd,gd->bsg', x, wi) + bi + bh
        Hh = wh.shape[1]

        def step(carry, g_t):
            h, c = carry
            g = g_t + h @ wh.T
            i, f, gg, o = jnp.split(g, 4, axis=-1)
            c = jax.nn.sigmoid(f) * c + jax.nn.sigmoid(i) * jnp.tanh(gg)
            h = jax.nn.sigmoid(o) * jnp.tanh(c)
            return (h, c), h

        init = (jnp.zeros((x.shape[0], Hh), x.dtype),
                jnp.zeros((x.shape[0], Hh), x.dtype))
        _, hs = jax.lax.scan(step, init, jnp.swapaxes(xg, 0, 1))
        hs = jnp.swapaxes(hs, 0, 1)
        if reverse:
            hs = hs[:, ::-1]
        return hs

    def _mp_full(v1, v2, w):
        a = v1[:, :, None, :] * w[None, None]
        if v2.ndim == 2:
            b = v2[:, None, None, :] * w[None, None]
        else:
            b = v2[:, :, None, :] * w[None, None]
        num = jnp.sum(a * b, axis=-1)
        den = jnp.linalg.norm(a, axis=-1) * jnp.linalg.norm(b, axis=-1)
        return num / jnp.maximum(den, EPS)

    def _mp_pair(v1, v2, w):
        a = w[None, :, None, :] * v1[:, None]
        b = w[None, :, None, :] * v2[:, None]
        n = jnp.einsum('blsh,blth->blst', a, b)
        d = jnp.linalg.norm(a, axis=-1)[..., None] * \
            jnp.linalg.norm(b, axis=-1)[:, :, None, :]
        return jnp.transpose(_div(n, d), (0, 2, 3, 1))

    def _attention(v1, v2):
        a = jnp.einsum('bsh,bth->bst', v1, v2)
        d = jnp.linalg.norm(v1, axis=-1)[:, :, None] * \
            jnp.linalg.norm(v2, axis=-1)[:, None, :]
        return _div(a, d)

    def phase_ctx(q1, q2, emb, cwi_f, cwh_f, cbi_f, cbh_f,
                  cwi_b, cwh_b, cbi_b, cbh_b):
        p = emb[q1]
        h = emb[q2]
        con_p_fw = _lstm(p, cwi_f, cwh_f, cbi_f, cbh_f)
        con_p_bw = _lstm(p, cwi_b, cwh_b, cbi_b, cbh_b, reverse=True)
        con_h_fw = _lstm(h, cwi_f, cwh_f, cbi_f, cbh_f)
        con_h_bw = _lstm(h, cwi_b, cwh_b, cbi_b, cbh_b, reverse=True)
        return con_p_fw, con_p_bw, con_h_fw, con_h_bw

    def phase_match(con_p_fw, con_p_bw, con_h_fw, con_h_bw, mp_w):
        mv_p_full_fw = _mp_full(con_p_fw, con_h_fw[:, -1, :], mp_w[0])
        mv_p_full_bw = _mp_full(con_p_bw, con_h_bw[:, 0, :], mp_w[1])
        mv_h_full_fw = _mp_full(con_h_fw, con_p_fw[:, -1, :], mp_w[0])
        mv_h_full_bw = _mp_full(con_h_bw, con_p_bw[:, 0, :], mp_w[1])

        mv_max_fw = _mp_pair(con_p_fw, con_h_fw, mp_w[2])
        mv_max_bw = _mp_pair(con_p_bw, con_h_bw, mp_w[3])
        mv_p_max_fw = mv_max_fw.max(axis=2)
        mv_p_max_bw = mv_max_bw.max(axis=2)
        mv_h_max_fw = mv_max_fw.max(axis=1)
        mv_h_max_bw = mv_max_bw.max(axis=1)

        att_fw = _attention(con_p_fw, con_h_fw)
        att_bw = _attention(con_p_bw, con_h_bw)
        att_h_fw = con_h_fw[:, None, :, :] * att_fw[:, :, :, None]
        att_h_bw = con_h_bw[:, None, :, :] * att_bw[:, :, :, None]
        att_p_fw = con_p_fw[:, :, None, :] * att_fw[:, :, :, None]
        att_p_bw = con_p_bw[:, :, None, :] * att_bw[:, :, :, None]

        att_mean_h_fw = _div(att_h_fw.sum(axis=2),
                             att_fw.sum(axis=2, keepdims=True))
        att_mean_h_bw = _div(att_h_bw.sum(axis=2),
                             att_bw.sum(axis=2, keepdims=True))
        att_mean_p_fw = _div(att_p_fw.sum(axis=1),
                             jnp.sum(att_fw, axis=1)[:, :, None])
        att_mean_p_bw = _div(att_p_bw.sum(axis=1),
                             jnp.sum(att_bw, axis=1)[:, :, None])

        mv_p_att_mean_fw = _mp_full(con_p_fw, att_mean_h_fw, mp_w[4])
        mv_p_att_mean_bw = _mp_full(con_p_bw, att_mean_h_bw, mp_w[5])
        mv_h_att_mean_fw = _mp_full(con_h_fw, att_mean_p_fw, mp_w[4])
        mv_h_att_mean_bw = _mp_full(con_h_bw, att_mean_p_bw, mp_w[5])

        att_max_h_fw = att_h_fw.max(axis=2)
        att_max_h_bw = att_h_bw.max(axis=2)
        att_max_p_fw = att_p_fw.max(axis=1)
        att_max_p_bw = att_p_bw.max(axis=1)

        mv_p_att_max_fw = _mp_full(con_p_fw, att_max_h_fw, mp_w[6])
        mv_p_att_max_bw = _mp_full(con_p_bw, att_max_h_bw, mp_w[7])
        mv_h_att_max_fw = _mp_full(con_h_fw, att_max_p_fw, mp_w[6])
        mv_h_att_max_bw = _mp_full(con_h_bw, att_max_p_bw, mp_w[7])

        mv_p = jnp.concatenate(
            [mv_p_full_fw, mv_p_max_fw, mv_p_att_mean_fw, mv_p_att_max_fw,
             mv_p_full_bw, mv_p_max_bw, mv_p_att_mean_bw, mv_p_att_max_bw],
            axis=2)
        mv_h = jnp.concatenate(
            [mv_h_full_fw, mv_h_max_fw, mv_h_att_mean_fw, mv_h_att_max_fw,
             mv_h_full_bw, mv_h_max_bw, mv_h_att_mean_bw, mv_h_att_max_bw],
            axis=2)
        return mv_p, mv_h

    def phase_agg(mv_p, mv_h, awi_f, awh_f, abi_f, abh_f,
                  awi_b, awh_b, abi_b, abh_b, w1, b1, w2, b2):
        hp_fw = _lstm(mv_p, awi_f, awh_f, abi_f, abh_f)[:, -1]
        hp_bw = _lstm(mv_p, awi_b, awh_b, abi_b, abh_b, reverse=True)[:, 0]
        hh_fw = _lstm(mv_h, awi_f, awh_f, abi_f, abh_f)[:, -1]
        hh_bw = _lstm(mv_h, awi_b, awh_b, abi_b, abh_b, reverse=True)[:, 0]
        x = jnp.concatenate([hp_fw, hp_bw, hh_fw, hh_bw], axis=1)
        x = jnp.tanh(x @ w1.T + b1)
        return x @ w2.T + b2

    return jax, jnp, phase_ctx, phase_match, phase_agg


def _get_state():
    if _STATE:
        return _STATE
    jax, jnp, phase_ctx, phase_match, phase_agg = _build()
    try:
        devs = [d for d in jax.devices() if d.platform != 'cpu'][:N_CORES]
    except Exception:
        devs = []
    n_dev = len(devs) if len(devs) == N_CORES else 0
    cpu = jax.devices('cpu')[0]

    _STATE.update(dict(
        jax=jax, jnp=jnp, n_dev=n_dev, devs=devs, cpu=cpu,
        fns=dict(ctx=phase_ctx, match=phase_match, agg=phase_agg),
        pmapped={}, cpu_jit={}, use_dev={}))

    if n_dev:
        _STATE['pmapped'] = {
            'ctx': jax.pmap(phase_ctx, in_axes=(0, 0) + (None,) * 9,
                            devices=devs),
            'match': jax.pmap(phase_match, in_axes=(0, 0, 0, 0, None),
                              devices=devs),
            'agg': jax.pmap(phase_agg, in_axes=(0, 0) + (None,) * 12,
                            devices=devs),
        }
    _STATE['cpu_jit'] = {
        'ctx': jax.jit(jax.vmap(phase_ctx, in_axes=(0, 0) + (None,) * 9),
                       device=cpu),
        'match': jax.jit(jax.vmap(phase_match, in_axes=(0, 0, 0, 0, None)),
                         device=cpu),
        'agg': jax.jit(jax.vmap(phase_agg, in_axes=(0, 0) + (None,) * 12),
                       device=cpu),
    }
    return _STATE


def _run_phase(name, sharded_args, weight_args):
    """Run a phase on the neuron cores, falling back to host on failure.

    sharded_args: tuple of arrays with leading (n_dev, B_loc, ...) layout
    weight_args: replicated arrays
    Returns tuple of outputs in (n_dev, B_loc, ...) layout (numpy or jax).
    """
    st = _get_state()
    use_dev = st['use_dev']
    if st['n_dev'] and use_dev.get(name, True):
        try:
            out = st['pmapped'][name](*sharded_args, *weight_args)
            use_dev[name] = True
            return out
        except Exception:
            use_dev[name] = False
    # host fallback: collapse shard axis into batch, vmap over batch
    flat = [np.asarray(a).reshape((-1,) + np.asarray(a).shape[2:])
            for a in sharded_args]
    out = st['cpu_jit'][name](*flat, *weight_args)
    if not isinstance(out, tuple):
        out = (out,)
    res = tuple(np.asarray(o).reshape((N_CORES, -1) + np.asarray(o).shape[1:])
                for o in out)
    return res if len(res) > 1 else res[0]


def kernel(q1, q2, emb, cwi_f, cwh_f, cbi_f, cbh_f, cwi_b, cwh_b, cbi_b,
           cbh_b, mp_w, awi_f, awh_f, abi_f, abh_f, awi_b, awh_b, abi_b,
           abh_b, w1, b1, w2, b2):
    _get_state()

    q1 = np.asarray(q1).astype(np.int32).reshape(N_CORES, B // N_CORES, S)
    q2 = np.asarray(q2).astype(np.int32).reshape(N_CORES, B // N_CORES, S)
    f32 = lambda x: np.asarray(x, dtype=np.float32)

    ctx_w = (f32(emb), f32(cwi_f), f32(cwh_f), f32(cbi_f), f32(cbh_f),
             f32(cwi_b), f32(cwh_b), f32(cbi_b), f32(cbh_b))
    agg_w = (f32(awi_f), f32(awh_f), f32(abi_f), f32(abh_f),
             f32(awi_b), f32(awh_b), f32(abi_b), f32(abh_b),
             f32(w1), f32(b1), f32(w2), f32(b2))

    cons = _run_phase('ctx', (q1, q2), ctx_w)
    mvs = _run_phase('match', tuple(cons), (f32(mp_w),))
    logits = _run_phase('agg', tuple(mvs), agg_w)

    logits = np.asarray(logits).reshape(B, C).astype(np.float32)
    pred = np.argmax(logits, axis=-1).astype(np.int32)
    return logits, pred


# revision 4
# speedup vs baseline: 80.4607x; 80.4607x over previous
"""BIMPM forward kernel for 8 Trainium2 NeuronCores.

Strategy (per sharding hint): data-parallel over batch. The 32-element batch
is split into 8 shards of 4; embedding/LSTM/matching/aggregation weights are
replicated on every core; all matching ops are batch-local, so there is no
cross-core communication. The forward pass is split into three pmapped
phases (context BiLSTM / multi-perspective matching / aggregation BiLSTM +
classifier head) compiled separately for the neuron backend; any phase whose
device compile fails falls back to host execution so the kernel always
produces correct output. The final (32,2) argmax runs on host.

Hardcoded problem dims: B=32, S=64, V=40000, E=300, H=256, L=20, C=2.
"""

import numpy as np

B, S, V, E, H, L, C = 32, 64, 40000, 300, 256, 20, 2
N_CORES = 8
EPS = 1e-8

_STATE = {}


def _build():
    import jax
    import jax.numpy as jnp

    def _div(n, d):
        d = jnp.where(d > EPS, d, EPS)
        return n / d

    def _lstm(x, wi, wh, bi, bh, reverse=False):
        if reverse:
            x = x[:, ::-1]
        xg = jnp.einsum('bsd,gd->bsg', x, wi) + bi + bh
        Hh = wh.shape[1]

        def step(carry, g_t):
            h, c = carry
            g = g_t + h @ wh.T
            i, f, gg, o = jnp.split(g, 4, axis=-1)
            c = jax.nn.sigmoid(f) * c + jax.nn.sigmoid(i) * jnp.tanh(gg)
            h = jax.nn.sigmoid(o) * jnp.tanh(c)
            return (h, c), h

        init = (jnp.zeros((x.shape[0], Hh), x.dtype),
                jnp.zeros((x.shape[0], Hh), x.dtype))
        _, hs = jax.lax.scan(step, init, jnp.swapaxes(xg, 0, 1))
        hs = jnp.swapaxes(hs, 0, 1)
        if reverse:
            hs = hs[:, ::-1]
        return hs

    def _mp_full(v1, v2, w):
        a = v1[:, :, None, :] * w[None, None]
        if v2.ndim == 2:
            b = v2[:, None, None, :] * w[None, None]
        else:
            b = v2[:, :, None, :] * w[None, None]
        num = jnp.sum(a * b, axis=-1)
        den = jnp.linalg.norm(a, axis=-1) * jnp.linalg.norm(b, axis=-1)
        return num / jnp.maximum(den, EPS)

    def _mp_pair(v1, v2, w):
        a = w[None, :, None, :] * v1[:, None]
        b = w[None, :, None, :] * v2[:, None]
        n = jnp.einsum('blsh,blth->blst', a, b)
        d = jnp.linalg.norm(a, axis=-1)[..., None] * \
            jnp.linalg.norm(b, axis=-1)[:, :, None, :]
        return jnp.transpose(_div(n, d), (0, 2, 3, 1))

    def _attention(v1, v2):
        a = jnp.einsum('bsh,bth->bst', v1, v2)
        d = jnp.linalg.norm(v1, axis=-1)[:, :, None] * \
            jnp.linalg.norm(v2, axis=-1)[:, None, :]
        return _div(a, d)

    def phase_ctx(q1, q2, emb, cwi_f, cwh_f, cbi_f, cbh_f,
                  cwi_b, cwh_b, cbi_b, cbh_b):
        p = emb[q1]
        h = emb[q2]
        con_p_fw = _lstm(p, cwi_f, cwh_f, cbi_f, cbh_f)
        con_p_bw = _lstm(p, cwi_b, cwh_b, cbi_b, cbh_b, reverse=True)
        con_h_fw = _lstm(h, cwi_f, cwh_f, cbi_f, cbh_f)
        con_h_bw = _lstm(h, cwi_b, cwh_b, cbi_b, cbh_b, reverse=True)
        return con_p_fw, con_p_bw, con_h_fw, con_h_bw

    def phase_match(con_p_fw, con_p_bw, con_h_fw, con_h_bw, mp_w):
        mv_p_full_fw = _mp_full(con_p_fw, con_h_fw[:, -1, :], mp_w[0])
        mv_p_full_bw = _mp_full(con_p_bw, con_h_bw[:, 0, :], mp_w[1])
        mv_h_full_fw = _mp_full(con_h_fw, con_p_fw[:, -1, :], mp_w[0])
        mv_h_full_bw = _mp_full(con_h_bw, con_p_bw[:, 0, :], mp_w[1])

        mv_max_fw = _mp_pair(con_p_fw, con_h_fw, mp_w[2])
        mv_max_bw = _mp_pair(con_p_bw, con_h_bw, mp_w[3])
        mv_p_max_fw = mv_max_fw.max(axis=2)
        mv_p_max_bw = mv_max_bw.max(axis=2)
        mv_h_max_fw = mv_max_fw.max(axis=1)
        mv_h_max_bw = mv_max_bw.max(axis=1)

        att_fw = _attention(con_p_fw, con_h_fw)
        att_bw = _attention(con_p_bw, con_h_bw)
        att_h_fw = con_h_fw[:, None, :, :] * att_fw[:, :, :, None]
        att_h_bw = con_h_bw[:, None, :, :] * att_bw[:, :, :, None]
        att_p_fw = con_p_fw[:, :, None, :] * att_fw[:, :, :, None]
        att_p_bw = con_p_bw[:, :, None, :] * att_bw[:, :, :, None]

        att_mean_h_fw = _div(att_h_fw.sum(axis=2),
                             att_fw.sum(axis=2, keepdims=True))
        att_mean_h_bw = _div(att_h_bw.sum(axis=2),
                             att_bw.sum(axis=2, keepdims=True))
        att_mean_p_fw = _div(att_p_fw.sum(axis=1),
                             jnp.sum(att_fw, axis=1)[:, :, None])
        att_mean_p_bw = _div(att_p_bw.sum(axis=1),
                             jnp.sum(att_bw, axis=1)[:, :, None])

        mv_p_att_mean_fw = _mp_full(con_p_fw, att_mean_h_fw, mp_w[4])
        mv_p_att_mean_bw = _mp_full(con_p_bw, att_mean_h_bw, mp_w[5])
        mv_h_att_mean_fw = _mp_full(con_h_fw, att_mean_p_fw, mp_w[4])
        mv_h_att_mean_bw = _mp_full(con_h_bw, att_mean_p_bw, mp_w[5])

        att_max_h_fw = att_h_fw.max(axis=2)
        att_max_h_bw = att_h_bw.max(axis=2)
        att_max_p_fw = att_p_fw.max(axis=1)
        att_max_p_bw = att_p_bw.max(axis=1)

        mv_p_att_max_fw = _mp_full(con_p_fw, att_max_h_fw, mp_w[6])
        mv_p_att_max_bw = _mp_full(con_p_bw, att_max_h_bw, mp_w[7])
        mv_h_att_max_fw = _mp_full(con_h_fw, att_max_p_fw, mp_w[6])
        mv_h_att_max_bw = _mp_full(con_h_bw, att_max_p_bw, mp_w[7])

        mv_p = jnp.concatenate(
            [mv_p_full_fw, mv_p_max_fw, mv_p_att_mean_fw, mv_p_att_max_fw,
             mv_p_full_bw, mv_p_max_bw, mv_p_att_mean_bw, mv_p_att_max_bw],
            axis=2)
        mv_h = jnp.concatenate(
            [mv_h_full_fw, mv_h_max_fw, mv_h_att_mean_fw, mv_h_att_max_fw,
             mv_h_full_bw, mv_h_max_bw, mv_h_att_mean_bw, mv_h_att_max_bw],
            axis=2)
        return mv_p, mv_h

    def phase_agg(mv_p, mv_h, awi_f, awh_f, abi_f, abh_f,
                  awi_b, awh_b, abi_b, abh_b, w1, b1, w2, b2):
        hp_fw = _lstm(mv_p, awi_f, awh_f, abi_f, abh_f)[:, -1]
        hp_bw = _lstm(mv_p, awi_b, awh_b, abi_b, abh_b, reverse=True)[:, 0]
        hh_fw = _lstm(mv_h, awi_f, awh_f, abi_f, abh_f)[:, -1]
        hh_bw = _lstm(mv_h, awi_b, awh_b, abi_b, abh_b, reverse=True)[:, 0]
        x = jnp.concatenate([hp_fw, hp_bw, hh_fw, hh_bw], axis=1)
        x = jnp.tanh(x @ w1.T + b1)
        return x @ w2.T + b2

    return jax, jnp, phase_ctx, phase_match, phase_agg


def _get_state():
    if _STATE:
        return _STATE
    jax, jnp, phase_ctx, phase_match, phase_agg = _build()
    try:
        devs = [d for d in jax.devices() if d.platform != 'cpu'][:N_CORES]
    except Exception:
        devs = []
    n_dev = len(devs) if len(devs) == N_CORES else 0
    cpu = jax.devices('cpu')[0]

    _STATE.update(dict(
        jax=jax, jnp=jnp, n_dev=n_dev, devs=devs, cpu=cpu,
        fns=dict(ctx=phase_ctx, match=phase_match, agg=phase_agg),
        pmapped={}, cpu_jit={}, use_dev={}))

    if n_dev:
        # all args carry an explicit leading device axis; weights are
        # replicated onto the cores once via device_put_replicated and
        # cached, so repeat calls do no H2D weight traffic.
        _STATE['pmapped'] = {
            'ctx': jax.pmap(phase_ctx, devices=devs),
            'match': jax.pmap(phase_match, devices=devs),
            'agg': jax.pmap(phase_agg, devices=devs),
        }
    _STATE['cpu_jit'] = {
        'ctx': jax.jit(jax.vmap(phase_ctx, in_axes=(0, 0) + (None,) * 9),
                       device=cpu),
        'match': jax.jit(jax.vmap(phase_match, in_axes=(0, 0, 0, 0, None)),
                         device=cpu),
        'agg': jax.jit(jax.vmap(phase_agg, in_axes=(0, 0) + (None,) * 12),
                       device=cpu),
    }
    return _STATE


def _replicated(name, arrs):
    """device_put_replicated `arrs` onto the cores, cached across calls.

    Re-validates cheaply against the cached host copies so a caller passing
    different weights gets a fresh replication rather than stale values.
    """
    st = _get_state()
    cache = st.setdefault('wcache', {})
    ent = cache.get(name)
    if ent is not None:
        host, dev = ent
        if len(host) == len(arrs) and all(
                h.shape == a.shape and np.array_equal(h, a)
                for h, a in zip(host, arrs)):
            return dev
    dev = [st['jax'].device_put_replicated(a, st['devs']) for a in arrs]
    cache[name] = ([np.asarray(a) for a in arrs], dev)
    return dev


def _run_phase(name, sharded_args, weight_args):
    """Run a phase on the neuron cores, falling back to host on failure.

    sharded_args: tuple of arrays with leading (n_dev, B_loc, ...) layout
    weight_args: replicated arrays
    Returns tuple of outputs in (n_dev, B_loc, ...) layout (numpy or jax).
    """
    st = _get_state()
    use_dev = st['use_dev']
    if st['n_dev'] and use_dev.get(name, True):
        try:
            wdev = _replicated(name, weight_args)
            out = st['pmapped'][name](*sharded_args, *wdev)
            use_dev[name] = True
            return out
        except Exception:
            use_dev[name] = False
    # host fallback: collapse shard axis into batch, vmap over batch
    flat = [np.asarray(a).reshape((-1,) + np.asarray(a).shape[2:])
            for a in sharded_args]
    out = st['cpu_jit'][name](*flat, *weight_args)
    if not isinstance(out, tuple):
        out = (out,)
    res = tuple(np.asarray(o).reshape((N_CORES, -1) + np.asarray(o).shape[1:])
                for o in out)
    return res if len(res) > 1 else res[0]


def kernel(q1, q2, emb, cwi_f, cwh_f, cbi_f, cbh_f, cwi_b, cwh_b, cbi_b,
           cbh_b, mp_w, awi_f, awh_f, abi_f, abh_f, awi_b, awh_b, abi_b,
           abh_b, w1, b1, w2, b2):
    _get_state()

    q1 = np.asarray(q1).astype(np.int32).reshape(N_CORES, B // N_CORES, S)
    q2 = np.asarray(q2).astype(np.int32).reshape(N_CORES, B // N_CORES, S)
    f32 = lambda x: np.asarray(x, dtype=np.float32)

    ctx_w = (f32(emb), f32(cwi_f), f32(cwh_f), f32(cbi_f), f32(cbh_f),
             f32(cwi_b), f32(cwh_b), f32(cbi_b), f32(cbh_b))
    agg_w = (f32(awi_f), f32(awh_f), f32(abi_f), f32(abh_f),
             f32(awi_b), f32(awh_b), f32(abi_b), f32(abh_b),
             f32(w1), f32(b1), f32(w2), f32(b2))

    cons = _run_phase('ctx', (q1, q2), ctx_w)
    mvs = _run_phase('match', tuple(cons), (f32(mp_w),))
    logits = _run_phase('agg', tuple(mvs), agg_w)

    logits = np.asarray(logits).reshape(B, C).astype(np.float32)
    pred = np.argmax(logits, axis=-1).astype(np.int32)
    return logits, pred


# revision 5
# speedup vs baseline: 150.3929x; 1.8691x over previous
"""BIMPM forward kernel for 8 Trainium2 NeuronCores.

Strategy (per sharding hint): data-parallel over batch. The 32-element batch
is split into 8 shards of 4; embedding/LSTM/matching/aggregation weights are
replicated on every core; all matching ops are batch-local, so there is no
cross-core communication. The forward pass is split into three pmapped
phases (context BiLSTM / multi-perspective matching / aggregation BiLSTM +
classifier head) compiled separately for the neuron backend; any phase whose
device compile fails falls back to host execution so the kernel always
produces correct output. The final (32,2) argmax runs on host.

Hardcoded problem dims: B=32, S=64, V=40000, E=300, H=256, L=20, C=2.
"""

import numpy as np

B, S, V, E, H, L, C = 32, 64, 40000, 300, 256, 20, 2
N_CORES = 8
EPS = 1e-8

_STATE = {}


def _build():
    import jax
    import jax.numpy as jnp
    try:
        jax.config.update('jax_compilation_cache_dir', '/tmp/jax_neuron_cache')
        jax.config.update('jax_persistent_cache_min_compile_time_secs', 1.0)
    except Exception:
        pass

    def _div(n, d):
        d = jnp.where(d > EPS, d, EPS)
        return n / d

    def _lstm(x, wi, wh, bi, bh, reverse=False):
        if reverse:
            x = x[:, ::-1]
        xg = jnp.einsum('bsd,gd->bsg', x, wi) + bi + bh
        Hh = wh.shape[1]

        def step(carry, g_t):
            h, c = carry
            g = g_t + h @ wh.T
            i, f, gg, o = jnp.split(g, 4, axis=-1)
            c = jax.nn.sigmoid(f) * c + jax.nn.sigmoid(i) * jnp.tanh(gg)
            h = jax.nn.sigmoid(o) * jnp.tanh(c)
            return (h, c), h

        init = (jnp.zeros((x.shape[0], Hh), x.dtype),
                jnp.zeros((x.shape[0], Hh), x.dtype))
        _, hs = jax.lax.scan(step, init, jnp.swapaxes(xg, 0, 1))
        hs = jnp.swapaxes(hs, 0, 1)
        if reverse:
            hs = hs[:, ::-1]
        return hs

    def _mp_full(v1, v2, w):
        a = v1[:, :, None, :] * w[None, None]
        if v2.ndim == 2:
            b = v2[:, None, None, :] * w[None, None]
        else:
            b = v2[:, :, None, :] * w[None, None]
        num = jnp.sum(a * b, axis=-1)
        den = jnp.linalg.norm(a, axis=-1) * jnp.linalg.norm(b, axis=-1)
        return num / jnp.maximum(den, EPS)

    def _mp_pair(v1, v2, w):
        a = w[None, :, None, :] * v1[:, None]
        b = w[None, :, None, :] * v2[:, None]
        n = jnp.einsum('blsh,blth->blst', a, b)
        d = jnp.linalg.norm(a, axis=-1)[..., None] * \
            jnp.linalg.norm(b, axis=-1)[:, :, None, :]
        return jnp.transpose(_div(n, d), (0, 2, 3, 1))

    def _attention(v1, v2):
        a = jnp.einsum('bsh,bth->bst', v1, v2)
        d = jnp.linalg.norm(v1, axis=-1)[:, :, None] * \
            jnp.linalg.norm(v2, axis=-1)[:, None, :]
        return _div(a, d)

    def phase_ctx(q1, q2, emb, cwi_f, cwh_f, cbi_f, cbh_f,
                  cwi_b, cwh_b, cbi_b, cbh_b):
        p = emb[q1]
        h = emb[q2]
        con_p_fw = _lstm(p, cwi_f, cwh_f, cbi_f, cbh_f)
        con_p_bw = _lstm(p, cwi_b, cwh_b, cbi_b, cbh_b, reverse=True)
        con_h_fw = _lstm(h, cwi_f, cwh_f, cbi_f, cbh_f)
        con_h_bw = _lstm(h, cwi_b, cwh_b, cbi_b, cbh_b, reverse=True)
        return con_p_fw, con_p_bw, con_h_fw, con_h_bw

    def phase_match(con_p_fw, con_p_bw, con_h_fw, con_h_bw, mp_w):
        mv_p_full_fw = _mp_full(con_p_fw, con_h_fw[:, -1, :], mp_w[0])
        mv_p_full_bw = _mp_full(con_p_bw, con_h_bw[:, 0, :], mp_w[1])
        mv_h_full_fw = _mp_full(con_h_fw, con_p_fw[:, -1, :], mp_w[0])
        mv_h_full_bw = _mp_full(con_h_bw, con_p_bw[:, 0, :], mp_w[1])

        mv_max_fw = _mp_pair(con_p_fw, con_h_fw, mp_w[2])
        mv_max_bw = _mp_pair(con_p_bw, con_h_bw, mp_w[3])
        mv_p_max_fw = mv_max_fw.max(axis=2)
        mv_p_max_bw = mv_max_bw.max(axis=2)
        mv_h_max_fw = mv_max_fw.max(axis=1)
        mv_h_max_bw = mv_max_bw.max(axis=1)

        att_fw = _attention(con_p_fw, con_h_fw)
        att_bw = _attention(con_p_bw, con_h_bw)
        att_h_fw = con_h_fw[:, None, :, :] * att_fw[:, :, :, None]
        att_h_bw = con_h_bw[:, None, :, :] * att_bw[:, :, :, None]
        att_p_fw = con_p_fw[:, :, None, :] * att_fw[:, :, :, None]
        att_p_bw = con_p_bw[:, :, None, :] * att_bw[:, :, :, None]

        att_mean_h_fw = _div(att_h_fw.sum(axis=2),
                             att_fw.sum(axis=2, keepdims=True))
        att_mean_h_bw = _div(att_h_bw.sum(axis=2),
                             att_bw.sum(axis=2, keepdims=True))
        att_mean_p_fw = _div(att_p_fw.sum(axis=1),
                             jnp.sum(att_fw, axis=1)[:, :, None])
        att_mean_p_bw = _div(att_p_bw.sum(axis=1),
                             jnp.sum(att_bw, axis=1)[:, :, None])

        mv_p_att_mean_fw = _mp_full(con_p_fw, att_mean_h_fw, mp_w[4])
        mv_p_att_mean_bw = _mp_full(con_p_bw, att_mean_h_bw, mp_w[5])
        mv_h_att_mean_fw = _mp_full(con_h_fw, att_mean_p_fw, mp_w[4])
        mv_h_att_mean_bw = _mp_full(con_h_bw, att_mean_p_bw, mp_w[5])

        att_max_h_fw = att_h_fw.max(axis=2)
        att_max_h_bw = att_h_bw.max(axis=2)
        att_max_p_fw = att_p_fw.max(axis=1)
        att_max_p_bw = att_p_bw.max(axis=1)

        mv_p_att_max_fw = _mp_full(con_p_fw, att_max_h_fw, mp_w[6])
        mv_p_att_max_bw = _mp_full(con_p_bw, att_max_h_bw, mp_w[7])
        mv_h_att_max_fw = _mp_full(con_h_fw, att_max_p_fw, mp_w[6])
        mv_h_att_max_bw = _mp_full(con_h_bw, att_max_p_bw, mp_w[7])

        mv_p = jnp.concatenate(
            [mv_p_full_fw, mv_p_max_fw, mv_p_att_mean_fw, mv_p_att_max_fw,
             mv_p_full_bw, mv_p_max_bw, mv_p_att_mean_bw, mv_p_att_max_bw],
            axis=2)
        mv_h = jnp.concatenate(
            [mv_h_full_fw, mv_h_max_fw, mv_h_att_mean_fw, mv_h_att_max_fw,
             mv_h_full_bw, mv_h_max_bw, mv_h_att_mean_bw, mv_h_att_max_bw],
            axis=2)
        return mv_p, mv_h

    def phase_agg(mv_p, mv_h, awi_f, awh_f, abi_f, abh_f,
                  awi_b, awh_b, abi_b, abh_b, w1, b1, w2, b2):
        hp_fw = _lstm(mv_p, awi_f, awh_f, abi_f, abh_f)[:, -1]
        hp_bw = _lstm(mv_p, awi_b, awh_b, abi_b, abh_b, reverse=True)[:, 0]
        hh_fw = _lstm(mv_h, awi_f, awh_f, abi_f, abh_f)[:, -1]
        hh_bw = _lstm(mv_h, awi_b, awh_b, abi_b, abh_b, reverse=True)[:, 0]
        x = jnp.concatenate([hp_fw, hp_bw, hh_fw, hh_bw], axis=1)
        x = jnp.tanh(x @ w1.T + b1)
        return x @ w2.T + b2

    return jax, jnp, phase_ctx, phase_match, phase_agg


def _get_state():
    if _STATE:
        return _STATE
    jax, jnp, phase_ctx, phase_match, phase_agg = _build()
    try:
        devs = [d for d in jax.devices() if d.platform != 'cpu'][:N_CORES]
    except Exception:
        devs = []
    n_dev = len(devs) if len(devs) == N_CORES else 0
    cpu = jax.devices('cpu')[0]

    _STATE.update(dict(
        jax=jax, jnp=jnp, n_dev=n_dev, devs=devs, cpu=cpu,
        fns=dict(ctx=phase_ctx, match=phase_match, agg=phase_agg),
        pmapped={}, cpu_jit={}, use_dev={}))

    if n_dev:
        # all args carry an explicit leading device axis; weights are
        # replicated onto the cores once via device_put_replicated and
        # cached, so repeat calls do no H2D weight traffic.
        _STATE['pmapped'] = {
            'ctx': jax.pmap(phase_ctx, devices=devs),
            'match': jax.pmap(phase_match, devices=devs),
            'agg': jax.pmap(phase_agg, devices=devs),
        }
    _STATE['cpu_jit'] = {
        'ctx': jax.jit(jax.vmap(phase_ctx, in_axes=(0, 0) + (None,) * 9),
                       device=cpu),
        'match': jax.jit(jax.vmap(phase_match, in_axes=(0, 0, 0, 0, None)),
                         device=cpu),
        'agg': jax.jit(jax.vmap(phase_agg, in_axes=(0, 0) + (None,) * 12),
                       device=cpu),
    }
    return _STATE


def _replicated(name, arrs):
    """device_put_replicated `arrs` onto the cores, cached across calls.

    Re-validates cheaply against the cached host copies so a caller passing
    different weights gets a fresh replication rather than stale values.
    """
    st = _get_state()
    cache = st.setdefault('wcache', {})
    ent = cache.get(name)
    if ent is not None:
        host, dev = ent
        if len(host) == len(arrs) and all(
                h.shape == a.shape and np.array_equal(h, a)
                for h, a in zip(host, arrs)):
            return dev
    dev = [st['jax'].device_put_replicated(a, st['devs']) for a in arrs]
    cache[name] = ([np.asarray(a) for a in arrs], dev)
    return dev


def _run_phase(name, sharded_args, weight_args):
    """Run a phase on the neuron cores, falling back to host on failure.

    sharded_args: tuple of arrays with leading (n_dev, B_loc, ...) layout
    weight_args: replicated arrays
    Returns tuple of outputs in (n_dev, B_loc, ...) layout (numpy or jax).
    """
    st = _get_state()
    use_dev = st['use_dev']
    if st['n_dev'] and use_dev.get(name, True):
        try:
            wdev = _replicated(name, weight_args)
            out = st['pmapped'][name](*sharded_args, *wdev)
            use_dev[name] = True
            return out
        except Exception:
            use_dev[name] = False
    # host fallback: collapse shard axis into batch, vmap over batch
    flat = [np.asarray(a).reshape((-1,) + np.asarray(a).shape[2:])
            for a in sharded_args]
    out = st['cpu_jit'][name](*flat, *weight_args)
    if not isinstance(out, tuple):
        out = (out,)
    res = tuple(np.asarray(o).reshape((N_CORES, -1) + np.asarray(o).shape[1:])
                for o in out)
    return res if len(res) > 1 else res[0]


def kernel(q1, q2, emb, cwi_f, cwh_f, cbi_f, cbh_f, cwi_b, cwh_b, cbi_b,
           cbh_b, mp_w, awi_f, awh_f, abi_f, abh_f, awi_b, awh_b, abi_b,
           abh_b, w1, b1, w2, b2):
    _get_state()

    q1 = np.asarray(q1).astype(np.int32).reshape(N_CORES, B // N_CORES, S)
    q2 = np.asarray(q2).astype(np.int32).reshape(N_CORES, B // N_CORES, S)
    f32 = lambda x: np.asarray(x, dtype=np.float32)

    ctx_w = (f32(emb), f32(cwi_f), f32(cwh_f), f32(cbi_f), f32(cbh_f),
             f32(cwi_b), f32(cwh_b), f32(cbi_b), f32(cbh_b))
    agg_w = (f32(awi_f), f32(awh_f), f32(abi_f), f32(abh_f),
             f32(awi_b), f32(awh_b), f32(abi_b), f32(abh_b),
             f32(w1), f32(b1), f32(w2), f32(b2))

    cons = _run_phase('ctx', (q1, q2), ctx_w)
    mvs = _run_phase('match', tuple(cons), (f32(mp_w),))
    logits = _run_phase('agg', tuple(mvs), agg_w)

    logits = np.asarray(logits).reshape(B, C).astype(np.float32)
    pred = np.argmax(logits, axis=-1).astype(np.int32)
    return logits, pred
